# revision 6
# baseline (speedup 1.0000x reference)
"""Trainium2 Bass kernel for DipolePredictorE3NN.

Reference computation (per batch b of B=1024, over N=4096 nodes):
    s      = feats[..., :5] @ w_path0                      # scalar gate
    tp     = C01 * s * edge + C11*w_path1[0] * cross(feats[...,5:8], edge)
    g      = tp.mean(nodes)                                # [B, 3]
    out    = relu(g @ W1 + b1) @ W2 + b2                   # [B, 3]

Strategy: data-parallel over batch, 8 cores x 128 batches; partition dim
= local batch (exactly 128), free dim = nodes, channel-planar per node
tile (host pre-arranges each core's shard so every on-chip operand is a
dense unit-stride plane).

v3 design (fp16 streams, DVE/ACT split, merged 3D-AP ops). Measured
facts driving it (v2 trace): DVE tensor_tensor runs 2x (0.52 ns/elem)
and tensor_scalar 4x (0.26) with 16-bit packed SBUF operands, ~155 ns
fixed per op; ACT reduces (activation Copy + accum_out) run 0.833
ns/elem with ~570 ns fixed per op (incl. the separate
ACTIVATION_READ_ACCUMULATOR); custom-DVE/STT/tensor_reduce are 1x, so
per-partition node sums belong on ACT. v2 used 4 tiles x 18 DVE + 9 ACT
ops and was ACT-fixed-cost-bound (63 us busy). v3:
  - 3 fat tiles [768, 1664, 1664] cut per-op fixed costs.
  - Host channel orders: feats ships [x0..x4, v1, v2, v0], edge ships
    [e2, e0, e1]. With (a,b) = (k+1, k+2) mod 3 this makes the +cross
    products (v_a*e_b) ONE 3-plane 3D-AP tensor_mul (in0 = f planes
    5:8, in1 = e planes 0:3, out planes k-ordered), and the s*e
    products one op too (in0 = s broadcast across planes, stride 0 -
    unit innermost stride keeps the 2x mode). -cross needs a wrapped
    permutation no AP can express -> 3 separate muls. Gate add tree is
    3 ops (one 2-plane 3D add + 2 plain). 13 DVE + 9 ACT ops/tile.
  - The s*e planes come out in e-plane order [2,0,1]; the per-plane ACT
    reduces just target the right pcol column, so the fold stays
    k-ordered.
  - feats tiles stream on the sync DMA queue, edge tiles + constants on
    the tensor-engine queue: two queues issue in parallel and the 4
    constant dma_starts stop delaying tile 1 (in v2 they cost 2.6 us of
    sync-queue issue time right in the ramp). The PE transpose identity
    is shipped from the host (make_identity burned 1.5 us of DVE before
    tile 0 in v2).
  - feats tile 0 is DMA'd in two pieces (x planes, then v planes) so
    the gate starts ~2 us before the v planes land.
Products stay UNSCALED fp16 (O(1) randn values; path constants fold
later in fp32): per-(tile,term) ACT accum cols -> one strided
tensor_reduce -> cross = cp - cm, g = pscale*se + c2*cross with
pscale = C01/N, c2 = C11*w1/N. g plus a ones column feeds a PE
transpose, then the MLP runs on the PE in transposed form (b1 folded
into the contraction, b2 via the activation bias): hT = relu(W1b^T .
[g|1]T), outT = W2^T . hT + b2. Per-core outT [3, 128]; the host
concatenates and transposes.
"""

import sys

if "/opt/trn_rl_repo" not in sys.path:
    sys.path.insert(0, "/opt/trn_rl_repo")

import numpy as np

C01 = float(np.sqrt(0.5) / np.sqrt(3.0))
C11 = float(np.sqrt(0.5) / np.sqrt(6.0))

B, N = 1024, 4096
NCORES = 8
BL = B // NCORES  # 128 local batches = partition count

TILES = [768, 1664, 1664]
assert sum(TILES) == N
TMAX = max(TILES)
NT = len(TILES)

# host-side channel orders (see docstring)
F_ORDER = [0, 1, 2, 3, 4, 6, 7, 5]  # on-chip planes 5,6,7 = v1, v2, v0
E_ORDER = [2, 0, 1]  # on-chip planes = e2, e0, e1
SE_K = [2, 0, 1]  # s*e plane j multiplies e_{SE_K[j]}

_CACHED = {}


def _build(w0_vals, w1_val):
    import concourse.bacc as bacc
    import concourse.mybir as mybir
    from concourse import tile

    f32 = mybir.dt.float32
    f16 = mybir.dt.float16
    Alu = mybir.AluOpType
    Act = mybir.ActivationFunctionType

    w0 = [float(w) for w in w0_vals]  # raw gate weights (s kept unscaled)
    pscale = C01 / float(N)  # applied at the fold
    c2 = C11 * float(w1_val) / float(N)  # applied at the fold

    nc = bacc.Bacc("TRN2", debug=False)

    feats = nc.dram_tensor("feats", [BL, 8 * N], f16, kind="ExternalInput").ap()
    edge = nc.dram_tensor("edge", [BL, 3 * N], f16, kind="ExternalInput").ap()
    ident = nc.dram_tensor("ident", [128, 128], f32, kind="ExternalInput").ap()
    W1 = nc.dram_tensor("W1", [3, 128], f32, kind="ExternalInput").ap()
    b1 = nc.dram_tensor("b1", [1, 128], f32, kind="ExternalInput").ap()
    W2 = nc.dram_tensor("W2", [128, 3], f32, kind="ExternalInput").ap()
    b2 = nc.dram_tensor("b2", [3, 1], f32, kind="ExternalInput").ap()
    outT = nc.dram_tensor("outT", [3, BL], f32, kind="ExternalOutput").ap()

    with tile.TileContext(nc) as tc:
        with (
            tc.tile_pool(name="consts", bufs=1) as consts,
            tc.tile_pool(name="state", bufs=1) as state,
            tc.tile_pool(name="fio", bufs=3) as fio,
            tc.tile_pool(name="eio", bufs=3) as eio,
            tc.tile_pool(name="gtmp", bufs=1) as gtmp,
            tc.tile_pool(name="sw", bufs=1) as sw,
            tc.tile_pool(name="cpp", bufs=2) as cpp,
            tc.tile_pool(name="cmp", bufs=2) as cmp_,
            tc.tile_pool(name="sep", bufs=2) as sep,
            tc.tile_pool(name="psum", bufs=1, space="PSUM") as psum,
        ):
            # all input streams issued up front: feats tiles on the sync
            # queue (tile 0 split x/v so the gate starts before the v
            # planes land), edge tiles + constants on the scalar queue
            # (issue cost lands before the first ACT reduce needs the
            # engine). Two queues stream in parallel.
            ftiles = []
            foff = 0
            for t, Tt in enumerate(TILES):
                ftile = fio.tile([128, 8 * TMAX], f16, tag="f", name=f"ftile{t}")
                if t == 0:
                    nc.sync.dma_start(
                        out=ftile[:, : 5 * Tt], in_=feats[:, foff : foff + 5 * Tt]
                    )
                    nc.sync.dma_start(
                        out=ftile[:, 5 * Tt : 8 * Tt],
                        in_=feats[:, foff + 5 * Tt : foff + 8 * Tt],
                    )
                else:
                    nc.sync.dma_start(
                        out=ftile[:, : 8 * Tt], in_=feats[:, foff : foff + 8 * Tt]
                    )
                ftiles.append(ftile)
                foff += 8 * Tt
            etiles = []
            eoff = 0
            for t, Tt in enumerate(TILES):
                etile = eio.tile([128, 3 * TMAX], f16, tag="e", name=f"etile{t}")
                nc.scalar.dma_start(
                    out=etile[:, : 3 * Tt], in_=edge[:, eoff : eoff + 3 * Tt]
                )
                etiles.append(etile)
                eoff += 3 * Tt

            # constants behind the e tiles; identity shipped, not generated
            identity = consts.tile([128, 128], f32)
            nc.scalar.dma_start(out=identity[:], in_=ident)
            w1b_s = consts.tile([4, 128], f32)
            nc.scalar.dma_start(out=w1b_s[0:3, :], in_=W1)
            nc.scalar.dma_start(out=w1b_s[3:4, :], in_=b1)
            w2_s = consts.tile([128, 3], f32)
            nc.scalar.dma_start(out=w2_s[:], in_=W2)
            b2_s = consts.tile([3, 1], f32)
            nc.scalar.dma_start(out=b2_s[:], in_=b2)

            # acc[:, 0:3] holds g; col 3 = 1.0 feeds the bias fold
            acc = state.tile([128, 4], f32)
            nc.vector.memset(acc[:, 3:4], 1.0)

            # per-(tile, term) fp32 partial sums, col t*9 + j with
            # j: 0-2 = +cross_k, 3-5 = -cross_k, 6-8 = (s*e)_k
            pcol = state.tile([128, NT * 9], f32)

            for t, Tt in enumerate(TILES):
                ftile, etile = ftiles[t], etiles[t]
                x = [ftile[:, u * Tt : (u + 1) * Tt] for u in range(8)]
                ev = [etile[:, k * Tt : (k + 1) * Tt] for k in range(3)]
                # 3D views [128, 3, Tt] over the channel planes
                f3 = ftile[:, 5 * Tt : 8 * Tt].rearrange("p (c n) -> p c n", c=3)
                e3 = etile[:, : 3 * Tt].rearrange("p (c n) -> p c n", c=3)

                # gate: 5 tensor_scalar muls (4x) + 3-op add tree (2x)
                tmp = gtmp.tile([128, 5 * TMAX], f16, tag="g", name="gatetmp")
                tp = [tmp[:, u * TMAX : u * TMAX + Tt] for u in range(5)]
                for u in range(5):
                    nc.vector.tensor_scalar_mul(tp[u], x[u], w0[u])
                # [t0, t2] += [t1, t3] as one 2-plane 3D add
                pair_out = tmp[:, 0 : 4 * TMAX].rearrange(
                    "p (c n) -> p c n", c=2
                )[:, :, 0:Tt]
                pair_in1 = tmp[:, TMAX : 5 * TMAX].rearrange(
                    "p (c n) -> p c n", c=2
                )[:, :, 0:Tt]
                nc.vector.tensor_add(pair_out, pair_out, pair_in1)
                nc.vector.tensor_add(tp[0], tp[0], tp[2])
                s_buf = sw.tile([128, TMAX], f16, tag="s", name="s_buf")
                nc.vector.tensor_add(s_buf[:, :Tt], tp[0], tp[4])

                # +cross: one 3-plane op, plane k = v_{k+1} * e_{k+2}
                cp = cpp.tile([128, 3 * TMAX], f16, tag="cp", name="cp")
                cp3 = cp[:].rearrange("p (c n) -> p c n", c=3)[:, :, 0:Tt]
                nc.vector.tensor_mul(cp3, f3[:, :, 0:Tt], e3[:, :, 0:Tt])
                # -cross: wrapped permutation -> 3 separate muls
                # cm_k = v_{k+2} * e_{k+1}; on-chip: (P6*E2, P7*E0, P5*E1)
                cm = cmp_.tile([128, 3 * TMAX], f16, tag="cm", name="cm")
                cmpl = [cm[:, k * TMAX : k * TMAX + Tt] for k in range(3)]
                nc.vector.tensor_mul(cmpl[0], x[6], ev[2])
                nc.vector.tensor_mul(cmpl[1], x[7], ev[0])
                nc.vector.tensor_mul(cmpl[2], x[5], ev[1])
                # s*e: one 3-plane op, in0 = s broadcast over planes
                se = sep.tile([128, 3 * TMAX], f16, tag="se", name="se")
                se3 = se[:].rearrange("p (c n) -> p c n", c=3)[:, :, 0:Tt]
                s3 = s_buf[:, 0:Tt].rearrange("p (c n) -> p c n", c=1)
                nc.vector.tensor_mul(se3, s3.broadcast_to((128, 3, Tt)), e3[:, :, 0:Tt])

                # ACT: 9 per-plane reduces into per-(tile,term) fp32 cols
                for k in range(3):
                    nc.scalar.activation(
                        cp[:, k * TMAX : k * TMAX + Tt],
                        cp[:, k * TMAX : k * TMAX + Tt],
                        Act.Copy,
                        accum_out=pcol[:, t * 9 + k : t * 9 + k + 1],
                    )
                for k in range(3):
                    nc.scalar.activation(
                        cm[:, k * TMAX : k * TMAX + Tt],
                        cm[:, k * TMAX : k * TMAX + Tt],
                        Act.Copy,
                        accum_out=pcol[:, t * 9 + 3 + k : t * 9 + 4 + k],
                    )
                for j in range(3):
                    k = SE_K[j]
                    nc.scalar.activation(
                        se[:, j * TMAX : j * TMAX + Tt],
                        se[:, j * TMAX : j * TMAX + Tt],
                        Act.Copy,
                        accum_out=pcol[:, t * 9 + 6 + k : t * 9 + 7 + k],
                    )

            # --- fold partials (all tiny, fp32) ---
            r9 = state.tile([128, 9], f32)
            pcol3 = pcol[:].rearrange("p (t j) -> p j t", j=9)
            nc.vector.tensor_reduce(
                out=r9[:], in_=pcol3, axis=mybir.AxisListType.X, op=Alu.add
            )
            cross_r = state.tile([128, 3], f32)
            nc.vector.tensor_sub(cross_r[:], r9[:, 0:3], r9[:, 3:6])
            crossc = state.tile([128, 3], f32)
            nc.vector.tensor_scalar_mul(crossc[:], cross_r[:], c2)
            # g = pscale * (s*e sums) + c2 * cross
            nc.vector.scalar_tensor_tensor(
                acc[:, 0:3], r9[:, 6:9], pscale, crossc[:], Alu.mult, Alu.add
            )

            # --- gT = transpose([g|1]): [128, 4] -> [4, 128] via PE ---
            gT_ps = psum.tile([4, 128], f32)
            nc.tensor.transpose(gT_ps[:], acc[:], identity[:])
            gT = state.tile([4, 128], f32)
            nc.scalar.copy(gT[:], gT_ps[:])

            # --- hT = relu(W1b^T(k,m) contracted with gT(k,n)) ---
            h_ps = psum.tile([128, 128], f32)
            nc.tensor.matmul(h_ps[:], lhsT=w1b_s[:], rhs=gT[:], start=True, stop=True)
            hT = state.tile([128, 128], f32)
            nc.scalar.activation(hT[:], h_ps[:], Act.Relu)

            # --- outT = W2^T . hT + b2 ---
            o_ps = psum.tile([3, 128], f32)
            nc.tensor.matmul(o_ps[:], lhsT=w2_s[:], rhs=hT[:], start=True, stop=True)
            oT = state.tile([3, 128], f32)
            nc.scalar.activation(oT[:], o_ps[:], Act.Identity, bias=b2_s[:])
            nc.sync.dma_start(out=outT, in_=oT[:])

    nc.finalize()
    return nc


def _get_nc(w_path0, w_path1):
    key = (
        np.asarray(w_path0, np.float32).tobytes(),
        np.asarray(w_path1, np.float32).tobytes(),
    )
    if _CACHED.get("key") != key:
        _CACHED["nc"] = _build(
            np.asarray(w_path0, np.float32).reshape(5),
            float(np.asarray(w_path1, np.float32).reshape(1)[0]),
        )
        _CACHED["key"] = key
    return _CACHED["nc"]


def _tile_major(shard, order):
    """[BL, N, C] fp16 -> [BL, sum_t C*Tt], channel-planar per tile in
    the given channel order."""
    C = len(order)
    blocks = []
    off = 0
    for Tt in TILES:
        blk = (
            shard[:, off : off + Tt, :][:, :, order]
            .transpose(0, 2, 1)
            .reshape(BL, C * Tt)
        )
        blocks.append(blk)
        off += Tt
    return np.ascontiguousarray(np.concatenate(blocks, axis=1))


def _in_maps(feats, edge_attr, W1, b1, W2, b2):
    f32 = np.float32
    identm = np.ascontiguousarray(np.eye(128, dtype=f32))
    W1m = np.ascontiguousarray(W1, f32).reshape(3, 128)
    b1m = np.ascontiguousarray(b1, f32).reshape(1, 128)
    W2m = np.ascontiguousarray(W2, f32).reshape(128, 3)
    b2m = np.ascontiguousarray(b2, f32).reshape(3, 1)
    feats = np.asarray(feats, np.float16)
    edge_attr = np.asarray(edge_attr, np.float16)
    maps = []
    for c in range(NCORES):
        sl = slice(c * BL, (c + 1) * BL)
        maps.append(
            {
                "feats": _tile_major(feats[sl], F_ORDER),
                "edge": _tile_major(edge_attr[sl], E_ORDER),
                "ident": identm,
                "W1": W1m,
                "b1": b1m,
                "W2": W2m,
                "b2": b2m,
            }
        )
    return maps


def run(inputs, trace=False, tmpdir=None):
    """Run on 8 cores; returns (out [B,3], BassKernelResults)."""
    from concourse import bass_utils

    nc = _get_nc(inputs["w_path0"], inputs["w_path1"])
    maps = _in_maps(
        inputs["feats"], inputs["edge_attr"],
        inputs["W1"], inputs["b1"], inputs["W2"], inputs["b2"],
    )
    kw = {}
    if trace:
        kw.update(trace=True, tmpdir=tmpdir)
    res = bass_utils.run_bass_kernel_spmd(
        nc, maps, core_ids=list(range(NCORES)), **kw
    )
    outT_full = np.concatenate([r["outT"] for r in res.results], axis=1)  # [3, B]
    return np.ascontiguousarray(outT_full.T), res


def kernel(feats, edge_attr, w_path0, w_path1, W1, b1, W2, b2):
    out, _ = run(
        dict(
            feats=feats, edge_attr=edge_attr, w_path0=w_path0, w_path1=w_path1,
            W1=W1, b1=b1, W2=W2, b2=b2,
        )
    )
    return out
